# revision 57
# baseline (speedup 1.0000x reference)
"""GQA attention (B=2, S=2048, DM=1024, H=16, KH=4, RoPE, causal) on 8 TRN2 cores.

Sharding: DP=2 over batch x TP=4 over heads. Core c handles batch c//4 and
q-heads [4r, 4r+4), kv-head r, where r = c % 4. Each core computes a partial
out^T = wo_shard @ attn_shard of shape [DM, S] in bf16; the host sums the 4
partials per batch in f32 and transposes (gather/unshard).

Per-core kernel (single NEFF, SPMD):
  - Q/K feature-major via transposed weight layouts prepared on host; V
    PE-transposed to token-major with a ones column appended (rowsum trick).
  - RoPE: adjacent-partition swap via stream_shuffle + elementwise ops.
  - Causal mask applied POST-exp: the upper triangle of the diagonal 128x128
    block of p2 is zeroed by a DVE multiply with a 0/1 triangle, so the
    rowsum (ones column) still gives the right denominator and the PE does
    no mask matmuls.
  - Scores matmuls run two heads in the two 64-row PE groups; diagonal key
    blocks only compute the unmasked column range. AV runs as a single
    matmul over both heads (shared V stationary) and lags one key-block so
    the PE never head-of-line blocks on the current exp. The PE clock (HAM)
    is pre-warmed with dummy matmuls while the first DMAs land.
  - Softmax division: denominator row reshaped across partitions via two
    small DMAs, DVE approx reciprocal, gpsimd partition_broadcast +
    multiplies; the cross-partition copy of the odd head rides the gpsimd
    SWDGE queue to stay off the congested store queues.
  - Projection and out-projection work is chopped into small "filler" items
    that are emitted between attention key-blocks so the PE never runs a
    long non-attention burst while the ACT engine starves. The final
    chunk's out-projection starts its pair-0 accumulation in repurposed
    score-PSUM slots while the last softmax divide is still in flight.
"""

from collections import deque

import numpy as np
import ml_dtypes

import concourse.bass as bass
import concourse.mybir as mybir
import concourse.tile as tile
from concourse import bacc
from concourse.bass_utils import run_bass_kernel_spmd
from concourse.masks import make_identity

F32 = mybir.dt.float32
BF16 = mybir.dt.bfloat16

B, S, DM, H, KH, HD = 2, 2048, 1024, 16, 4, 64
N_CORES = 8
TPG = 4                 # tensor-parallel group size
QH = H // TPG           # q-heads per core
KFEAT = QH * HD         # 256 q-features per core
SC = 512                # token chunk
NCH = S // SC           # 4
KB = 128                # key block
NKB = S // KB           # 16
SCALE = 1.0 / np.sqrt(HD)
XOR1 = [i ^ 1 for i in range(32)]

LAST_RESULTS = None     # BassKernelResults of the most recent run (for test.py)
_NC_CACHE = None


def build_nc():
    nc = bacc.Bacc("TRN2", target_bir_lowering=False, debug=False,
                   num_devices=1)

    # all inputs pre-packed on host so each partition line is contiguous
    xP = nc.declare_dram_parameter("xP", [128, NCH, 8, SC], BF16, isOutput=False)
    wqP = nc.declare_dram_parameter("wqP", [2, 128, 8, 128], BF16, isOutput=False)
    wkvP = nc.declare_dram_parameter("wkvP", [128, 8, 128], BF16, isOutput=False)
    woP = nc.declare_dram_parameter("woP", [128, 2, DM], BF16, isOutput=False)
    ropeCos = nc.declare_dram_parameter("ropeCos", [128, S], BF16, isOutput=False)
    ropeSin = nc.declare_dram_parameter("ropeSin", [128, S], BF16, isOutput=False)
    triM = nc.declare_dram_parameter("triM", [128, 2, 128], BF16, isOutput=False)
    out = nc.declare_dram_parameter("out", [128, NCH, 8, SC], BF16, isOutput=True)

    EXP = mybir.ActivationFunctionType.Exp
    MUL = bass.mybir.AluOpType.mult
    ADD = bass.mybir.AluOpType.add

    with tile.TileContext(nc) as tc:
        with (
            tc.tile_pool(name="consts", bufs=1) as consts,
            tc.tile_pool(name="kch", bufs=NCH) as kch_pool,
            tc.tile_pool(name="qch", bufs=NCH) as qch_pool,
            tc.tile_pool(name="ach", bufs=2 * NCH) as ach_pool,
            tc.tile_pool(name="v1p", bufs=NKB) as v1_pool,
            tc.tile_pool(name="xch", bufs=NCH) as xch_pool,
            tc.tile_pool(name="tmp", bufs=3) as tmp_pool,
            tc.tile_pool(name="pp", bufs=8) as p_pool,
            tc.tile_pool(name="ocp", bufs=3) as oc_pool,
            tc.tile_pool(name="rp", bufs=2) as r_pool,
            tc.tile_pool(name="bcp", bufs=4) as bc_pool,
            tc.tile_pool(name="op", bufs=2) as o_pool,
            tc.tile_pool(name="acc", bufs=2, space="PSUM") as acc_pool,
            tc.tile_pool(name="oac", bufs=1, space="PSUM") as oacc_pool,
            tc.tile_pool(name="sme", bufs=2, space="PSUM") as s_pool,
        ):
            # ---- constants (ordered for startup latency) ----
            wq_m = [consts.tile([128, 8, 128], BF16, tag=f"wqm{m}",
                                name=f"wq_m{m}") for m in range(2)]
            wkv_sb = consts.tile([128, 8, 128], BF16, tag="wkv")
            wo_sb = consts.tile([128, 2, DM], BF16, tag="wo")
            cos_sb = consts.tile([128, S], BF16, tag="cos")
            sin_sb = consts.tile([128, S], BF16, tag="sin")
            tri_sb = consts.tile([128, 2, 128], BF16, tag="tri")
            ident = consts.tile([128, 128], BF16, tag="ident")
            ones_b = consts.tile([128, 64], BF16, tag="onesb")

            def load_x(c0, eng=None, split=False):
                xt = xch_pool.tile([128, 8, SC], BF16, tag="x",
                                   name=f"x_c{c0}")
                e = eng or nc.sync
                if split:
                    # pieces alternate HWDGE queues -> parallel transfer and
                    # fine-grained readiness for the per-half projection MMs
                    for g in range(4):
                        eng2 = nc.scalar if g % 2 == 0 else nc.sync
                        eng2.dma_start(xt[:, 2 * g:2 * g + 2, :],
                                       xP[:, c0, 2 * g:2 * g + 2, :])
                else:
                    e.dma_start(xt[:], xP[:, c0, :, :])
                return xt

            with tc.high_priority():
                # order: only what the first chunk's KV/Q-m0 projections and
                # RoPE need; everything else (wq-m1, wo) is deferred so the
                # head is not DMA-bandwidth starved
                nc.sync.dma_start(wkv_sb[:], wkvP[:])
                X_ch = [load_x(0, split=True)]
                QS = S // 4
                nc.sync.dma_start(wq_m[0][:], wqP[0])
                nc.scalar.dma_start(tri_sb[:], triM[:])
                nc.sync.dma_start(cos_sb[:, 0:QS], ropeCos[:, 0:QS])
                nc.scalar.dma_start(sin_sb[:, 0:QS], ropeSin[:, 0:QS])
                make_identity(nc, ident[:])
                nc.vector.memset(ones_b[:], 1.0)
                # warm up the PE clock (HAM) while the first DMAs land:
                # back-to-back dummy matmuls on the identity tile
                wm_ps = acc_pool.tile([128, 128], F32, tag="acc",
                                      name="warm")
                for _ in range(52):
                    nc.tensor.matmul(wm_ps[:], ident[:], ident[:],
                                     start=True, stop=True)
            def load_rope_rest():
                # remaining cos/sin quarters arrive during chunk-0
                # attention, well before chunk-1 projections need them;
                # issued late so the chunk-0 K-dup doesn't queue behind them
                for qq in range(1, 4):
                    nc.sync.dma_start(cos_sb[:, qq * QS:(qq + 1) * QS],
                                      ropeCos[:, qq * QS:(qq + 1) * QS])
                    nc.scalar.dma_start(sin_sb[:, qq * QS:(qq + 1) * QS],
                                        ropeSin[:, qq * QS:(qq + 1) * QS])

            K_ch = []       # per-chunk K, feature-major, duplicated rows
            Q_ch = []       # per-chunk Q, feature-major, [128, 2, SC]
            A_ch = []       # per-chunk normalized attn tiles (2 pairs)
            V1_kb = []      # per key-block token-major [V | 1]

            filler = deque()

            def emit_filler(n=1):
                for _ in range(n):
                    if filler:
                        filler.popleft()()

            def drain_filler():
                while filler:
                    filler.popleft()()

            def proj_q_items(c0, xt, ms=(0, 1), rope_eng=None):
                """Queue Q projection + RoPE for chunk c0 as filler items.

                rope_eng: engine for the RoPE muls/add (cast+shuffle always
                DVE -- gpsimd can't read PSUM and has no shuffle). gpsimd
                here lets the chunk-0 Q rope run in parallel with the DVE
                K-RoPE chain that gates the first scores matmul."""
                re_ = rope_eng or nc.vector
                cols = slice(c0 * SC, (c0 + 1) * SC)
                if ms[0] == 0:
                    q_sb = qch_pool.tile([128, 2, SC], BF16, tag="q",
                                         name=f"q{c0}")
                    Q_ch.append(q_sb)
                else:
                    q_sb = Q_ch[c0]
                for m in ms:
                    q_ps = acc_pool.tile([128, SC], F32, tag="acc",
                                         name=f"qps{c0}m{m}")
                    box = {}

                    def mm_half(goff, m=m, q_ps=q_ps):
                        for g in range(4):
                            nc.tensor.matmul(
                                q_ps[:],
                                wq_m[m][:, goff + g, :],
                                xt[:, goff + g, :],
                                start=(goff + g == 0), stop=(goff + g == 7))

                    def rope_a(m=m, q_ps=q_ps, box=box):
                        # cast once to bf16 so shuffle+muls+add run at 2x DVE
                        # rate (stream_shuffle needs matching src/dst dtypes)
                        qb = tmp_pool.tile([128, SC], BF16, tag="qb")
                        qsw = tmp_pool.tile([128, SC], BF16, tag="qsw")
                        t1 = tmp_pool.tile([128, SC], BF16, tag="t1")
                        nc.vector.tensor_copy(qb[:], q_ps[:])
                        nc.vector.stream_shuffle(qsw[:], qb[:], XOR1)
                        re_.tensor_tensor(t1[:], qb[:],
                                          cos_sb[:, cols], MUL)
                        box["qsw"], box["t1"] = qsw, t1

                    def rope_b(m=m, box=box):
                        qsw, t1 = box["qsw"], box["t1"]
                        t2 = tmp_pool.tile([128, SC], BF16, tag="t2")
                        re_.tensor_tensor(t2[:], qsw[:],
                                          sin_sb[:, cols], MUL)
                        re_.tensor_tensor(q_sb[:, m, :], t1[:], t2[:],
                                          ADD)

                    filler.append(lambda mm_half=mm_half: mm_half(0))
                    filler.append(lambda mm_half=mm_half: mm_half(4))
                    filler.append(rope_a)
                    filler.append(rope_b)

            def proj_kv_items(c0, xt):
                """Queue K/V projection for chunk c0 as filler items."""
                cols = slice(c0 * SC, (c0 + 1) * SC)
                k_sb = kch_pool.tile([128, SC], BF16, tag="k", name=f"k{c0}")
                K_ch.append(k_sb)
                kv_ps = acc_pool.tile([128, SC], F32, tag="acc",
                                      name=f"kvps{c0}")
                v1s = []
                for tb in range(4):
                    v1 = v1_pool.tile([128, 66], BF16, tag="v1",
                                      name=f"v1_{c0}_{tb}")
                    v1s.append(v1)
                    V1_kb.append(v1)
                box = {}

                def mm_half(goff):
                    for g in range(4):
                        nc.tensor.matmul(
                            kv_ps[:], wkv_sb[:, goff + g, :],
                            xt[:, goff + g, :],
                            start=(goff + g == 0), stop=(goff + g == 7))

                def krope_a():
                    kbc = tmp_pool.tile([64, SC], BF16, tag="kbc")
                    ksw = tmp_pool.tile([64, SC], BF16, tag="ksw")
                    t1k = tmp_pool.tile([64, SC], BF16, tag="t1k")
                    nc.vector.tensor_copy(kbc[:], kv_ps[0:64, :])
                    nc.vector.stream_shuffle(ksw[:], kbc[:], XOR1)
                    nc.vector.tensor_tensor(t1k[:], kbc[:],
                                            cos_sb[0:64, cols], MUL)
                    box["ksw"], box["t1k"] = ksw, t1k

                def krope_b():
                    ksw, t1k = box["ksw"], box["t1k"]
                    t2k = tmp_pool.tile([64, SC], BF16, tag="t2k")
                    nc.vector.tensor_tensor(t2k[:], ksw[:],
                                            sin_sb[0:64, cols], MUL)
                    nc.vector.tensor_tensor(k_sb[0:64, :], t1k[:], t2k[:], ADD)
                    # duplicate K to partitions 64:128 via the PE (identity
                    # into column group 64:127) + a DVE copy; an SBUF-to-SBUF
                    # DMA here queues behind bulk loads in the rings and can
                    # stall the head-1 scores by microseconds
                    kd_ps = acc_pool.tile([128, SC], F32, tag="acc",
                                          name=f"kdup{c0}")
                    nc.tensor.matmul(kd_ps[64:128, :], ident[0:64, 0:64],
                                     k_sb[0:64, :], start=True, stop=True,
                                     tile_position=(0, 64))
                    nc.scalar.copy(k_sb[64:128, :], kd_ps[64:128, :])

                def vcast():
                    vtmp = tmp_pool.tile([128, SC], BF16, tag="vtmp")
                    nc.vector.tensor_copy(vtmp[64:128, :], kv_ps[64:128, :])
                    box["vtmp"] = vtmp

                def vtrans(tb2):
                    vtmp = box["vtmp"]
                    for tb in (tb2, tb2 + 1):
                        v1 = v1s[tb]
                        vt_ps = acc_pool.tile([128, 64], BF16, tag="acc")
                        nc.tensor.transpose(
                            vt_ps[:], vtmp[64:128, tb * 128:(tb + 1) * 128],
                            ident[64:128, 64:128])
                        nc.vector.tensor_copy(v1[:, 0:64], vt_ps[:])
                        nc.vector.memset(v1[:, 64:65], 1.0)

                filler.append(lambda: mm_half(0))
                filler.append(lambda: mm_half(4))
                filler.append(krope_a)
                filler.append(krope_b)
                filler.append(vcast)
                filler.append(lambda: vtrans(0))
                filler.append(lambda: vtrans(2))

            def out_proj_items(c0, hold_tail=False, act_casts=False):
                """Queue the out-projection of chunk c0 as filler items."""
                osb = o_pool.tile([128, 8, SC], BF16, tag="osb",
                                  name=f"osb{c0}")

                def mb_item(mb, on_act=False):
                    o_ps = acc_pool.tile([128, SC], F32, tag="acc")
                    for c in range(2):
                        nc.tensor.matmul(
                            o_ps[:], wo_sb[:, c, mb * 128:(mb + 1) * 128],
                            A_ch[c0][c][:, :], start=(c == 0), stop=(c == 1))
                    if on_act:
                        nc.scalar.copy(osb[:, mb, :], o_ps[:])
                    else:
                        nc.vector.tensor_copy(osb[:, mb, :], o_ps[:])

                # split the store so the first half overlaps the second's MMs
                alt = hold_tail or act_casts
                items = [lambda mb=mb, a=alt: mb_item(mb, a and mb % 2 == 1)
                         for mb in range(8)]
                items.insert(4, lambda: nc.scalar.dma_start(
                    out[:, c0, 0:4, :], osb[:, 0:4, :]))
                items.append(lambda: nc.sync.dma_start(
                    out[:, c0, 4:8, :], osb[:, 4:8, :]))
                if hold_tail:
                    filler.extend(items[:5])
                    return items[5:]
                filler.extend(items)
                return []

            def attention_pair(c0, p):
                nkb = 4 * (c0 + 1)
                nfill = 3 if c0 == 0 else (2 if c0 == 1 else 1)
                op2 = oacc_pool.tile([65, 2, SC], F32, tag="op2")
                pend = None     # (kb, q0, p2) waiting for its AV matmul

                def av(kb, q0, p2):
                    # per-head matmuls: a single merged one would need a
                    # >1-PSUM-bank output, which the ISA forbids
                    nc.tensor.matmul(
                        op2[:, 0, q0:], V1_kb[kb][:, 0:65], p2[:, 0, q0:],
                        start=(kb == 0), stop=(kb == nkb - 1))
                    nc.tensor.matmul(
                        op2[:, 1, q0:], V1_kb[kb][:, 0:65], p2[:, 1, q0:],
                        start=(kb == 0), stop=(kb == nkb - 1))

                for kb in range(nkb):
                    kc = K_ch[kb // 4]
                    kcols = slice((kb % 4) * 128, (kb % 4 + 1) * 128)
                    j = kb - (nkb - 4)
                    q0 = 128 * j if j > 0 else 0
                    s2 = s_pool.tile([128, 2, SC], F32, tag="s2")
                    nc.tensor.matmul(
                        s2[:, 0, q0:], kc[0:64, kcols],
                        Q_ch[c0][0:64, p, q0:],
                        start=True, stop=True)
                    nc.tensor.matmul(
                        s2[:, 1, q0:], kc[64:128, kcols],
                        Q_ch[c0][64:128, p, q0:],
                        start=True, stop=True, tile_position=(64, 0))
                    p2 = p_pool.tile([128, 2, SC], BF16, tag="p2")
                    nc.scalar.activation(p2[:, :, q0:], s2[:, :, q0:],
                                         EXP, scale=SCALE)
                    if j >= 0:
                        # causal mask: zero the strict upper triangle of the
                        # diagonal 128-block post-exp (rowsum then excludes it)
                        nc.vector.tensor_tensor(
                            p2[:, :, q0:q0 + 128], p2[:, :, q0:q0 + 128],
                            tri_sb[:], MUL)
                    # AV lags one kb so the PE never head-of-line blocks on
                    # the exp of the current kb
                    if pend is not None:
                        av(*pend)
                    pend = (kb, q0, p2)
                    emit_filler(nfill)
                av(*pend)
                return op2

            def divide_pair(a_tile, op2, pad_clock=False):
                # evacuate numerator + denominators (frees PSUM banks)
                oc = oc_pool.tile([65, 2, SC], F32, tag="oc")
                nc.vector.tensor_copy(oc[:], op2[:])
                # reshape each [1, 512] sums row to [32, 16] so the
                # reciprocal runs on many DVE lanes instead of one
                rsum = r_pool.tile([64, 16], F32, tag="rsum")
                for hh in range(2):
                    nc.sync.dma_start(
                        rsum[32 * hh: 32 * hh + 32, :],
                        oc[64:65, hh, :].rearrange("o (a n) -> o a n", a=32))
                rrecs = r_pool.tile([64, 16], F32, tag="rrecs")
                nc.vector.reciprocal_approx_fast(rrecs[:], rsum[:])
                rrec = r_pool.tile([1, 2, SC], F32, tag="rrec")
                for hh in range(2):
                    nc.sync.dma_start(
                        rrec[0:1, hh, :].rearrange("o (a n) -> o a n", a=32),
                        rrecs[32 * hh: 32 * hh + 32, :])
                for hh in range(2):
                    bc = bc_pool.tile([64, SC], F32, tag="bc")
                    nc.gpsimd.partition_broadcast(bc[:], rrec[0:1, hh, :])
                    if hh == 0:
                        nc.vector.tensor_tensor(
                            a_tile[0:64, :], oc[0:64, hh, :], bc[:], MUL)
                    else:
                        tb = bc_pool.tile([64, SC], BF16, tag="tb")
                        nc.vector.tensor_tensor(
                            tb[:], oc[0:64, hh, :], bc[:], MUL)
                        # move to partitions 64:128 on the idle SWDGE queue
                        nc.gpsimd.dma_start(a_tile[64:128, :], tb[:])

            def divide_fast(a_tile, op2):
                """Latency-optimized divide for the final pair: PE rank-1
                broadcast of the bf16 denominator row, approx reciprocal,
                no small-DMA roundtrips in the chain."""
                oc = oc_pool.tile([65, 2, SC], BF16, tag="ocf")
                nc.vector.tensor_copy(oc[:], op2[:])
                pbcA = acc_pool.tile([64, SC], F32, tag="acc")
                pbcB = acc_pool.tile([64, SC], F32, tag="acc")
                nc.tensor.matmul(pbcA[:], ones_b[64:65, :], oc[64:65, 0, :],
                                 start=True, stop=True)
                nc.tensor.matmul(pbcB[:], ones_b[64:65, :], oc[64:65, 1, :],
                                 start=True, stop=True)
                rbcA = bc_pool.tile([64, SC], F32, tag="bc")
                rbcB = bc_pool.tile([64, SC], F32, tag="bc")
                nc.vector.reciprocal_approx_fast(rbcA[:], pbcA[:])
                nc.vector.reciprocal_approx_fast(rbcB[:], pbcB[:])
                nc.vector.tensor_tensor(a_tile[0:64, :], oc[0:64, 0, :],
                                        rbcA[:], MUL)
                tb = bc_pool.tile([64, SC], BF16, tag="tb")
                nc.vector.tensor_tensor(tb[:], oc[0:64, 1, :], rbcB[:], MUL)
                nc.gpsimd.dma_start(a_tile[64:128, :], tb[:])

            def final_out_proj(c0, divide_chain, a1):
                """Out-projection of the last chunk: pair-0 accumulations
                start in repurposed score-PSUM slots while the last divide
                is still in flight; the pair-1 A tile's odd head arrives via
                a cross-partition move on the quiet SWDGE queue."""
                osb = o_pool.tile([128, 8, SC], BF16, tag="osb",
                                  name=f"osb{c0}")
                # 4 o_ps slots carved out of the (now idle) score PSUM bufs
                s_a = s_pool.tile([128, 2, SC], F32, tag="s2", name="fop_a")
                s_b = s_pool.tile([128, 2, SC], F32, tag="s2", name="fop_b")
                slots = [s_a[:, 0, :], s_a[:, 1, :], s_b[:, 0, :], s_b[:, 1, :]]

                def mm0(mb, o_ps):
                    nc.tensor.matmul(
                        o_ps, wo_sb[:, 0, mb * 128:(mb + 1) * 128],
                        A_ch[c0][0][:, :], start=True, stop=False)

                def mm1(mb, o_ps):
                    nc.tensor.matmul(
                        o_ps, wo_sb[:, 1, mb * 128:(mb + 1) * 128],
                        a1[:], start=False, stop=True)

                # phase 1: pair-0 accumulation for mb0-5 (depends only on the
                # pair-0 divide, done long ago); divide chain ops interleave,
                # and dummy matmuls keep the PE clock from dropping while the
                # divide's DVE chain runs
                accs = []
                for mb in range(6):
                    if mb < 4:
                        o_ps = slots[mb]
                    else:
                        o_t = acc_pool.tile([128, SC], F32, tag="acc",
                                            name=f"fop_acc{mb}")
                        o_ps = o_t[:]
                    accs.append(o_ps)
                    mm0(mb, o_ps)
                    divide_chain(mb)
                # phase 2: pair-1 accumulation + casts + early stores
                for mb in range(6):
                    mm1(mb, accs[mb])
                    divide_chain(6 + mb)
                    if mb % 2 == 1:
                        nc.scalar.copy(osb[:, mb, :], accs[mb])
                        eng = nc.scalar if mb == 1 else nc.sync
                        eng.dma_start(out[:, c0, mb - 1:mb + 1, :],
                                      osb[:, mb - 1:mb + 1, :])
                    else:
                        nc.vector.tensor_copy(osb[:, mb, :], accs[mb])
                # phase 3: mb6-7 through the regular acc pool
                for mb in range(6, 8):
                    o_ps = acc_pool.tile([128, SC], F32, tag="acc")
                    mm0(mb, o_ps[:])
                    mm1(mb, o_ps[:])
                    if mb % 2 == 1:
                        nc.scalar.copy(osb[:, mb, :], o_ps[:])
                    else:
                        nc.vector.tensor_copy(osb[:, mb, :], o_ps[:])
                nc.sync.dma_start(out[:, c0, 6:8, :], osb[:, 6:8, :])

            # chunk 0: run only what attention pair 0 needs eagerly (KV, K
            # RoPE, Q-m0 + its RoPE); V transposes and Q-m1 go to the filler
            # so the first scores matmul fires as soon as the DMAs land
            proj_kv_items(0, X_ch[0])
            kvi = [filler.popleft() for _ in range(len(filler))]
            proj_q_items(0, X_ch[0], ms=(0,), rope_eng=nc.gpsimd)
            qi0 = [filler.popleft() for _ in range(len(filler))]
            # the KV->K-RoPE->dup chain gates the first scores matmul pair;
            # emit it high-priority so the compile-time scheduler doesn't
            # push the KV projection behind Q work
            with tc.high_priority():
                for it in (kvi[0], kvi[1], kvi[2], kvi[3]):
                    it()
            for it in (qi0[0], qi0[1], qi0[2], qi0[3]):
                it()
            # deferred loads: wq-m1 (needed mid-pair-0), wo (needed at the
            # first out-projection, a chunk later)
            nc.scalar.dma_start(wq_m[1][:], wqP[1])
            nc.scalar.dma_start(wo_sb[:], woP[:])
            filler.extend(kvi[4:7])                   # V cast + transposes
            proj_q_items(0, X_ch[0], ms=(1,))         # Q-m1 as filler
            xn = None
            pending_outproj = None
            for c0 in range(NCH):
                a_pair = [ach_pool.tile([128, SC], BF16, tag="a",
                                        name=f"a_c{c0}p{i}")
                          for i in range(2)]
                A_ch.append(a_pair)
                if c0 == 0:
                    X_ch.append(load_x(1))
                if c0 + 1 < NCH:
                    xn = X_ch[c0 + 1]
                    proj_q_items(c0 + 1, xn)
                op0 = attention_pair(c0, 0)
                if c0 == 0:
                    load_rope_rest()
                divide_pair(a_pair[0], op0, pad_clock=(c0 <= 1))
                if c0 + 1 < NCH:
                    proj_kv_items(c0 + 1, xn)
                held = []
                if pending_outproj is not None:
                    held = out_proj_items(pending_outproj,
                                          hold_tail=(c0 == NCH - 1),
                                          act_casts=True)
                op1 = attention_pair(c0, 1)
                for it in held:
                    it()
                if c0 == NCH - 1:
                    # final pair: latency-optimized divide interleaved with
                    # the final out-projection's pair-0 accumulation; both
                    # head-halves stay on partitions 0:64
                    steps = deque()
                    a1 = ach_pool.tile([128, SC], BF16, tag="a",
                                        name="a1f")

                    def queue_divide():
                        # only the denominator rows are evacuated (1-lane
                        # casts, split per head so head 0's chain starts
                        # sooner); the numerator multiplies read op1's PSUM
                        # directly -- nothing else needs those banks after
                        ocd = oc_pool.tile([1, 2, SC], BF16, tag="ocd")
                        nc.vector.tensor_copy(ocd[:], op1[64:65, :, :])
                        pbcA = acc_pool.tile([64, SC], F32, tag="acc")
                        pbcB = acc_pool.tile([64, SC], F32, tag="acc")
                        rbcA = bc_pool.tile([64, SC], F32, tag="bc")
                        rbcB = bc_pool.tile([64, SC], F32, tag="bc")
                        # dummy matmuls keep the PE clock up while the
                        # divide's DVE chain runs; reading ocd pins them in
                        # this window (the scheduler can't hoist them early),
                        # and they land in pbc banks that the real broadcasts
                        # overwrite right after
                        for i in range(12):
                            tgt = pbcA if i % 2 == 0 else pbcB
                            nc.tensor.matmul(tgt[:, 0:128], ones_b[0:1, :],
                                             ocd[0:1, 0, 0:128],
                                             start=True, stop=True)

                        def st1():
                            nc.tensor.matmul(pbcA[:], ones_b[0:1, :],
                                             ocd[0:1, 0, :],
                                             start=True, stop=True)
                            nc.tensor.matmul(pbcB[:], ones_b[0:1, :],
                                             ocd[0:1, 1, :],
                                             start=True, stop=True)

                        def st2():
                            nc.vector.reciprocal_approx_fast(rbcA[:], pbcA[:])
                            nc.vector.tensor_tensor(
                                a1[0:64, :], op1[0:64, 0, :], rbcA[:], MUL)
                            # WAR on pbcA orders these after the reciprocal:
                            # they fill the PE during the divide's DVE ops
                            for _ in range(6):
                                nc.tensor.matmul(pbcA[:, 0:128],
                                                 ones_b[0:1, :],
                                                 ocd[0:1, 0, 0:128],
                                                 start=True, stop=True)

                        def st3():
                            nc.vector.reciprocal_approx_fast(rbcB[:], pbcB[:])
                            tbf = bc_pool.tile([64, SC], BF16, tag="tb",
                                               name="tbf")
                            nc.vector.tensor_tensor(
                                tbf[:], op1[0:64, 1, :], rbcB[:], MUL)
                            # rings are quiet here; the SWDGE queue keeps
                            # this off the store-carrying HWDGE queues
                            nc.gpsimd.dma_start(a1[64:128, :], tbf[:])
                            for _ in range(6):
                                nc.tensor.matmul(pbcB[:, 0:128],
                                                 ones_b[0:1, :],
                                                 ocd[0:1, 0, 0:128],
                                                 start=True, stop=True)

                        steps.extend([st1, st2, st3])

                    queue_divide()

                    def divide_chain(i, steps=steps):
                        if steps:
                            steps.popleft()()

                    drain_filler()
                    final_out_proj(c0, divide_chain, a1)
                else:
                    divide_pair(a_pair[1], op1, pad_clock=(c0 <= 1))
                    if c0 + 2 < NCH:
                        X_ch.append(load_x(c0 + 2))
                    # Q/KV of chunk c0+1 must be ready before attention starts
                    drain_filler()
                    pending_outproj = c0

    nc.compile()
    return nc


def shard_inputs(x, wq, wk, wv, wo, freqs_cos, freqs_sin):
    """Build the 8 per-core input maps (host-side layout prep)."""
    x = np.ascontiguousarray(np.asarray(x, dtype=np.float32))
    wq = np.asarray(wq, dtype=np.float32)
    wk = np.asarray(wk, dtype=np.float32)
    wv = np.asarray(wv, dtype=np.float32)
    wo = np.asarray(wo, dtype=np.float32)
    cos = np.asarray(freqs_cos, dtype=np.float32)   # [S, 32]
    sin = np.asarray(freqs_sin, dtype=np.float32)
    bf = ml_dtypes.bfloat16

    rope_cos = np.repeat(cos.T, 2, axis=0)          # [64, S]
    rope_sin = np.repeat(sin.T, 2, axis=0)
    rope_sin[0::2, :] *= -1.0                       # row 2i: -sin_i, 2i+1: +sin_i
    rope_cos = np.ascontiguousarray(
        np.concatenate([rope_cos, rope_cos], 0)).astype(bf)
    rope_sin = np.ascontiguousarray(
        np.concatenate([rope_sin, rope_sin], 0)).astype(bf)

    # 0/1 keep-mask for a 128x128 diagonal block: 0 where k > q (strict
    # upper triangle), applied to p2 post-exp
    kk = np.arange(128)[:, None]
    qq = np.arange(128)[None, :]
    tri = np.where(kk > qq, np.float32(0.0), np.float32(1.0))
    triM = np.ascontiguousarray(
        np.broadcast_to(tri[:, None, :], (128, 2, 128))).astype(bf)

    in_maps = []
    for core in range(N_CORES):
        b, r = divmod(core, TPG)
        xT = x[b].T                                               # [DM, S]
        # pack so each SBUF partition line is one contiguous DRAM run
        xPm = np.ascontiguousarray(
            xT.reshape(8, 128, NCH, SC).transpose(1, 2, 0, 3))    # [128,NCH,8,SC]
        wq_s = wq[r * KFEAT:(r + 1) * KFEAT]                      # [256, DM]
        wk_s = wk[r * HD:(r + 1) * HD]                            # [64, DM]
        wv_s = wv[r * HD:(r + 1) * HD]
        wkvT = np.concatenate([wk_s, wv_s], axis=0).T             # [DM, 128]
        wqT = wq_s.T                                              # [DM, 256]
        woT = wo[:, r * KFEAT:(r + 1) * KFEAT].T                  # [256, DM]
        wqPm = np.ascontiguousarray(
            wqT.reshape(8, 128, 2, 128).transpose(2, 1, 0, 3))    # [2, 128, 8, 128]
        wkvPm = np.ascontiguousarray(
            wkvT.reshape(8, 128, 128).transpose(1, 0, 2))         # [128, 8, 128]
        woPm = np.ascontiguousarray(
            woT.reshape(2, 128, DM).transpose(1, 0, 2))           # [128, 2, 1024]
        in_maps.append({
            "xP": xPm.astype(bf),
            "wqP": wqPm.astype(bf),
            "wkvP": wkvPm.astype(bf),
            "woP": woPm.astype(bf),
            "ropeCos": rope_cos,
            "ropeSin": rope_sin,
            "triM": triM,
        })
    return in_maps


def unshard(results):
    """Sum TP partials per batch, unpack, and transpose to [B, S, DM]."""
    out = np.empty((B, S, DM), dtype=np.float32)
    for b in range(B):
        acc = results[b * TPG]["out"].astype(np.float32)
        for r in range(1, TPG):
            acc = acc + results[b * TPG + r]["out"].astype(np.float32)
        # [128, NCH, 8, SC] -> [DM, S]: row (mb*128+p), col (c*SC+n)
        full = acc.transpose(2, 0, 1, 3).reshape(DM, S)
        out[b] = full.T
    return out


def kernel(**inputs):
    global LAST_RESULTS, _NC_CACHE
    if _NC_CACHE is None:
        _NC_CACHE = build_nc()
    in_maps = shard_inputs(**inputs)
    LAST_RESULTS = run_bass_kernel_spmd(_NC_CACHE, in_maps, list(range(N_CORES)))
    return unshard(LAST_RESULTS.results)


# revision 59
# speedup vs baseline: 1.0534x; 1.0534x over previous
"""GQA attention (B=2, S=2048, DM=1024, H=16, KH=4, RoPE, causal) on 8 TRN2 cores.

Sharding: DP=2 over batch x TP=4 over heads. Core c handles batch c//4 and
q-heads [4r, 4r+4), kv-head r, where r = c % 4. Each core computes a partial
out^T = wo_shard @ attn_shard of shape [DM, S] in bf16; the host sums the 4
partials per batch in f32 and transposes (gather/unshard).

Per-core kernel (single NEFF, SPMD):
  - Q/K feature-major via transposed weight layouts prepared on host; V
    PE-transposed to token-major with a ones column appended (rowsum trick).
  - RoPE: adjacent-partition swap via stream_shuffle + elementwise ops.
  - Causal mask applied POST-exp: the upper triangle of the diagonal 128x128
    block of p2 is zeroed by a DVE multiply with a 0/1 triangle, so the
    rowsum (ones column) still gives the right denominator and the PE does
    no mask matmuls.
  - Scores matmuls run two heads in the two 64-row PE groups; diagonal key
    blocks only compute the unmasked column range. AV runs as a single
    matmul over both heads (shared V stationary) and lags one key-block so
    the PE never head-of-line blocks on the current exp. The PE clock (HAM)
    is pre-warmed with dummy matmuls while the first DMAs land.
  - Softmax division: denominator row reshaped across partitions via two
    small DMAs, DVE approx reciprocal, gpsimd partition_broadcast +
    multiplies; the cross-partition copy of the odd head rides the gpsimd
    SWDGE queue to stay off the congested store queues.
  - Projection and out-projection work is chopped into small "filler" items
    that are emitted between attention key-blocks so the PE never runs a
    long non-attention burst while the ACT engine starves. The final
    chunk's out-projection starts its pair-0 accumulation in repurposed
    score-PSUM slots while the last softmax divide is still in flight.
"""

from collections import deque

import numpy as np
import ml_dtypes

import concourse.bass as bass
import concourse.mybir as mybir
import concourse.tile as tile
from concourse import bacc
from concourse.bass_utils import run_bass_kernel_spmd
from concourse.masks import make_identity

F32 = mybir.dt.float32
BF16 = mybir.dt.bfloat16

B, S, DM, H, KH, HD = 2, 2048, 1024, 16, 4, 64
N_CORES = 8
TPG = 4                 # tensor-parallel group size
QH = H // TPG           # q-heads per core
KFEAT = QH * HD         # 256 q-features per core
SC = 512                # token chunk
NCH = S // SC           # 4
KB = 128                # key block
NKB = S // KB           # 16
SCALE = 1.0 / np.sqrt(HD)
XOR1 = [i ^ 1 for i in range(32)]

LAST_RESULTS = None     # BassKernelResults of the most recent run (for test.py)
_NC_CACHE = None


def build_nc():
    nc = bacc.Bacc("TRN2", target_bir_lowering=False, debug=False,
                   num_devices=1)

    # all inputs pre-packed on host so each partition line is contiguous
    xP = nc.declare_dram_parameter("xP", [128, NCH, 8, SC], BF16, isOutput=False)
    wqP = nc.declare_dram_parameter("wqP", [2, 128, 8, 128], BF16, isOutput=False)
    wkvP = nc.declare_dram_parameter("wkvP", [128, 8, 128], BF16, isOutput=False)
    woP = nc.declare_dram_parameter("woP", [128, 2, DM], BF16, isOutput=False)
    ropeCos = nc.declare_dram_parameter("ropeCos", [128, S], BF16, isOutput=False)
    ropeSin = nc.declare_dram_parameter("ropeSin", [128, S], BF16, isOutput=False)
    triM = nc.declare_dram_parameter("triM", [128, 2, 128], BF16, isOutput=False)
    out = nc.declare_dram_parameter("out", [128, NCH, 8, SC], BF16, isOutput=True)

    EXP = mybir.ActivationFunctionType.Exp
    MUL = bass.mybir.AluOpType.mult
    ADD = bass.mybir.AluOpType.add

    with tile.TileContext(nc) as tc:
        with (
            tc.tile_pool(name="consts", bufs=1) as consts,
            tc.tile_pool(name="kch", bufs=NCH) as kch_pool,
            tc.tile_pool(name="qch", bufs=NCH) as qch_pool,
            tc.tile_pool(name="ach", bufs=2 * NCH) as ach_pool,
            tc.tile_pool(name="v1p", bufs=NKB) as v1_pool,
            tc.tile_pool(name="xch", bufs=NCH) as xch_pool,
            tc.tile_pool(name="tmp", bufs=3) as tmp_pool,
            tc.tile_pool(name="pp", bufs=8) as p_pool,
            tc.tile_pool(name="ocp", bufs=3) as oc_pool,
            tc.tile_pool(name="rp", bufs=2) as r_pool,
            tc.tile_pool(name="bcp", bufs=4) as bc_pool,
            tc.tile_pool(name="op", bufs=2) as o_pool,
            tc.tile_pool(name="acc", bufs=2, space="PSUM") as acc_pool,
            tc.tile_pool(name="oac", bufs=1, space="PSUM") as oacc_pool,
            tc.tile_pool(name="sme", bufs=2, space="PSUM") as s_pool,
        ):
            # ---- constants (ordered for startup latency) ----
            wq_m = [consts.tile([128, 8, 128], BF16, tag=f"wqm{m}",
                                name=f"wq_m{m}") for m in range(2)]
            wkv_sb = consts.tile([128, 8, 128], BF16, tag="wkv")
            wo_sb = consts.tile([128, 2, DM], BF16, tag="wo")
            cos_sb = consts.tile([128, S], BF16, tag="cos")
            sin_sb = consts.tile([128, S], BF16, tag="sin")
            tri_sb = consts.tile([128, 2, 128], BF16, tag="tri")
            ident = consts.tile([128, 128], BF16, tag="ident")
            ones_b = consts.tile([128, 64], BF16, tag="onesb")

            def load_x(c0, eng=None, split=False):
                xt = xch_pool.tile([128, 8, SC], BF16, tag="x",
                                   name=f"x_c{c0}")
                e = eng or nc.sync
                if split:
                    # pieces alternate HWDGE queues -> parallel transfer and
                    # fine-grained readiness for the per-half projection MMs
                    for g in range(4):
                        eng2 = nc.scalar if g % 2 == 0 else nc.sync
                        eng2.dma_start(xt[:, 2 * g:2 * g + 2, :],
                                       xP[:, c0, 2 * g:2 * g + 2, :])
                else:
                    e.dma_start(xt[:], xP[:, c0, :, :])
                return xt

            with tc.high_priority():
                # order: only what the first chunk's KV/Q-m0 projections and
                # RoPE need; everything else (wq-m1, wo) is deferred so the
                # head is not DMA-bandwidth starved
                nc.sync.dma_start(wkv_sb[:], wkvP[:])
                X_ch = [load_x(0, split=True)]
                QS = S // 4
                nc.sync.dma_start(wq_m[0][:], wqP[0])
                nc.scalar.dma_start(tri_sb[:], triM[:])
                nc.sync.dma_start(cos_sb[:, 0:QS], ropeCos[:, 0:QS])
                nc.scalar.dma_start(sin_sb[:, 0:QS], ropeSin[:, 0:QS])
                make_identity(nc, ident[:])
                nc.vector.memset(ones_b[:], 1.0)
                # warm up the PE clock (HAM) while the first DMAs land:
                # back-to-back dummy matmuls on the identity tile
                wm_ps = acc_pool.tile([128, 128], F32, tag="acc",
                                      name="warm")
                for _ in range(52):
                    nc.tensor.matmul(wm_ps[:], ident[:], ident[:],
                                     start=True, stop=True)
            def load_rope_rest():
                # remaining cos/sin quarters arrive during chunk-0
                # attention, well before chunk-1 projections need them;
                # issued late so the chunk-0 K-dup doesn't queue behind them
                for qq in range(1, 4):
                    nc.sync.dma_start(cos_sb[:, qq * QS:(qq + 1) * QS],
                                      ropeCos[:, qq * QS:(qq + 1) * QS])
                    nc.scalar.dma_start(sin_sb[:, qq * QS:(qq + 1) * QS],
                                        ropeSin[:, qq * QS:(qq + 1) * QS])

            K_ch = []       # per-chunk K, feature-major, duplicated rows
            Q_ch = []       # per-chunk Q, feature-major, [128, 2, SC]
            A_ch = []       # per-chunk normalized attn tiles (2 pairs)
            V1_kb = []      # per key-block token-major [V | 1]

            filler = deque()

            def emit_filler(n=1):
                for _ in range(n):
                    if filler:
                        filler.popleft()()

            def drain_filler():
                while filler:
                    filler.popleft()()

            def proj_q_items(c0, xt, ms=(0, 1), rope_eng=None):
                """Queue Q projection + RoPE for chunk c0 as filler items.

                rope_eng: engine for the RoPE muls/add (cast+shuffle always
                DVE -- gpsimd can't read PSUM and has no shuffle). gpsimd
                here lets the chunk-0 Q rope run in parallel with the DVE
                K-RoPE chain that gates the first scores matmul."""
                re_ = rope_eng or nc.vector
                cols = slice(c0 * SC, (c0 + 1) * SC)
                if ms[0] == 0:
                    q_sb = qch_pool.tile([128, 2, SC], BF16, tag="q",
                                         name=f"q{c0}")
                    Q_ch.append(q_sb)
                else:
                    q_sb = Q_ch[c0]
                for m in ms:
                    q_ps = acc_pool.tile([128, SC], F32, tag="acc",
                                         name=f"qps{c0}m{m}")
                    box = {}

                    def mm_half(goff, m=m, q_ps=q_ps):
                        for g in range(4):
                            nc.tensor.matmul(
                                q_ps[:],
                                wq_m[m][:, goff + g, :],
                                xt[:, goff + g, :],
                                start=(goff + g == 0), stop=(goff + g == 7))

                    def rope_a(m=m, q_ps=q_ps, box=box):
                        # cast once to bf16 so shuffle+muls+add run at 2x DVE
                        # rate (stream_shuffle needs matching src/dst dtypes)
                        qb = tmp_pool.tile([128, SC], BF16, tag="qb")
                        qsw = tmp_pool.tile([128, SC], BF16, tag="qsw")
                        t1 = tmp_pool.tile([128, SC], BF16, tag="t1")
                        nc.vector.tensor_copy(qb[:], q_ps[:])
                        nc.vector.stream_shuffle(qsw[:], qb[:], XOR1)
                        re_.tensor_tensor(t1[:], qb[:],
                                          cos_sb[:, cols], MUL)
                        box["qsw"], box["t1"] = qsw, t1

                    def rope_b(m=m, box=box):
                        qsw, t1 = box["qsw"], box["t1"]
                        t2 = tmp_pool.tile([128, SC], BF16, tag="t2")
                        re_.tensor_tensor(t2[:], qsw[:],
                                          sin_sb[:, cols], MUL)
                        re_.tensor_tensor(q_sb[:, m, :], t1[:], t2[:],
                                          ADD)

                    filler.append(lambda mm_half=mm_half: mm_half(0))
                    filler.append(lambda mm_half=mm_half: mm_half(4))
                    filler.append(rope_a)
                    filler.append(rope_b)

            def proj_kv_items(c0, xt):
                """Queue K/V projection for chunk c0 as filler items."""
                cols = slice(c0 * SC, (c0 + 1) * SC)
                k_sb = kch_pool.tile([128, SC], BF16, tag="k", name=f"k{c0}")
                K_ch.append(k_sb)
                kv_ps = acc_pool.tile([128, SC], F32, tag="acc",
                                      name=f"kvps{c0}")
                v1s = []
                for tb in range(4):
                    v1 = v1_pool.tile([128, 66], BF16, tag="v1",
                                      name=f"v1_{c0}_{tb}")
                    v1s.append(v1)
                    V1_kb.append(v1)
                box = {}

                def mm_half(goff):
                    for g in range(4):
                        nc.tensor.matmul(
                            kv_ps[:], wkv_sb[:, goff + g, :],
                            xt[:, goff + g, :],
                            start=(goff + g == 0), stop=(goff + g == 7))

                def krope_a():
                    kbc = tmp_pool.tile([64, SC], BF16, tag="kbc")
                    ksw = tmp_pool.tile([64, SC], BF16, tag="ksw")
                    t1k = tmp_pool.tile([64, SC], BF16, tag="t1k")
                    nc.vector.tensor_copy(kbc[:], kv_ps[0:64, :])
                    nc.vector.stream_shuffle(ksw[:], kbc[:], XOR1)
                    nc.vector.tensor_tensor(t1k[:], kbc[:],
                                            cos_sb[0:64, cols], MUL)
                    box["ksw"], box["t1k"] = ksw, t1k

                def krope_b():
                    ksw, t1k = box["ksw"], box["t1k"]
                    t2k = tmp_pool.tile([64, SC], BF16, tag="t2k")
                    nc.vector.tensor_tensor(t2k[:], ksw[:],
                                            sin_sb[0:64, cols], MUL)
                    nc.vector.tensor_tensor(k_sb[0:64, :], t1k[:], t2k[:], ADD)
                    # duplicate K to partitions 64:128 via the PE (identity
                    # into column group 64:127) + a DVE copy; an SBUF-to-SBUF
                    # DMA here queues behind bulk loads in the rings and can
                    # stall the head-1 scores by microseconds
                    kd_ps = acc_pool.tile([128, SC], F32, tag="acc",
                                          name=f"kdup{c0}")
                    nc.tensor.matmul(kd_ps[64:128, :], ident[0:64, 0:64],
                                     k_sb[0:64, :], start=True, stop=True,
                                     tile_position=(0, 64))
                    nc.scalar.copy(k_sb[64:128, :], kd_ps[64:128, :])

                def vcast():
                    vtmp = tmp_pool.tile([128, SC], BF16, tag="vtmp")
                    nc.vector.tensor_copy(vtmp[64:128, :], kv_ps[64:128, :])
                    box["vtmp"] = vtmp

                def vtrans(tb2):
                    vtmp = box["vtmp"]
                    for tb in (tb2, tb2 + 1):
                        v1 = v1s[tb]
                        vt_ps = acc_pool.tile([128, 64], BF16, tag="acc")
                        nc.tensor.transpose(
                            vt_ps[:], vtmp[64:128, tb * 128:(tb + 1) * 128],
                            ident[64:128, 64:128])
                        nc.vector.tensor_copy(v1[:, 0:64], vt_ps[:])
                        nc.vector.memset(v1[:, 64:65], 1.0)

                filler.append(lambda: mm_half(0))
                filler.append(lambda: mm_half(4))
                filler.append(krope_a)
                filler.append(krope_b)
                filler.append(vcast)
                filler.append(lambda: vtrans(0))
                filler.append(lambda: vtrans(2))

            def out_proj_items(c0, hold_tail=False, act_casts=False):
                """Queue the out-projection of chunk c0 as filler items."""
                osb = o_pool.tile([128, 8, SC], BF16, tag="osb",
                                  name=f"osb{c0}")

                def mb_item(mb, on_act=False):
                    o_ps = acc_pool.tile([128, SC], F32, tag="acc")
                    for c in range(2):
                        nc.tensor.matmul(
                            o_ps[:], wo_sb[:, c, mb * 128:(mb + 1) * 128],
                            A_ch[c0][c][:, :], start=(c == 0), stop=(c == 1))
                    if on_act:
                        nc.scalar.copy(osb[:, mb, :], o_ps[:])
                    else:
                        nc.vector.tensor_copy(osb[:, mb, :], o_ps[:])

                # split the store so the first half overlaps the second's MMs
                alt = act_casts
                items = [lambda mb=mb, a=alt: mb_item(mb, a and mb % 2 == 1)
                         for mb in range(8)]
                items.insert(4, lambda: nc.scalar.dma_start(
                    out[:, c0, 0:4, :], osb[:, 0:4, :]))
                items.append(lambda: nc.sync.dma_start(
                    out[:, c0, 4:8, :], osb[:, 4:8, :]))
                if hold_tail:
                    filler.extend(items[:5])
                    return items[5:]
                filler.extend(items)
                return []

            def attention_pair(c0, p):
                nkb = 4 * (c0 + 1)
                nfill = 3 if c0 == 0 else (2 if c0 == 1 else 1)
                op2 = oacc_pool.tile([65, 2, SC], F32, tag="op2")
                pend = None     # (kb, q0, p2) waiting for its AV matmul

                def av(kb, q0, p2):
                    # per-head matmuls: a single merged one would need a
                    # >1-PSUM-bank output, which the ISA forbids
                    nc.tensor.matmul(
                        op2[:, 0, q0:], V1_kb[kb][:, 0:65], p2[:, 0, q0:],
                        start=(kb == 0), stop=(kb == nkb - 1))
                    nc.tensor.matmul(
                        op2[:, 1, q0:], V1_kb[kb][:, 0:65], p2[:, 1, q0:],
                        start=(kb == 0), stop=(kb == nkb - 1))

                for kb in range(nkb):
                    kc = K_ch[kb // 4]
                    kcols = slice((kb % 4) * 128, (kb % 4 + 1) * 128)
                    j = kb - (nkb - 4)
                    q0 = 128 * j if j > 0 else 0
                    s2 = s_pool.tile([128, 2, SC], F32, tag="s2")
                    nc.tensor.matmul(
                        s2[:, 0, q0:], kc[0:64, kcols],
                        Q_ch[c0][0:64, p, q0:],
                        start=True, stop=True)
                    nc.tensor.matmul(
                        s2[:, 1, q0:], kc[64:128, kcols],
                        Q_ch[c0][64:128, p, q0:],
                        start=True, stop=True, tile_position=(64, 0))
                    p2 = p_pool.tile([128, 2, SC], BF16, tag="p2")
                    nc.scalar.activation(p2[:, :, q0:], s2[:, :, q0:],
                                         EXP, scale=SCALE)
                    if j >= 0:
                        # causal mask: zero the strict upper triangle of the
                        # diagonal 128-block post-exp (rowsum then excludes it)
                        nc.vector.tensor_tensor(
                            p2[:, :, q0:q0 + 128], p2[:, :, q0:q0 + 128],
                            tri_sb[:], MUL)
                    # AV lags one kb so the PE never head-of-line blocks on
                    # the exp of the current kb
                    if pend is not None:
                        av(*pend)
                    pend = (kb, q0, p2)
                    emit_filler(nfill)
                av(*pend)
                return op2

            def divide_pair(a_tile, op2, pad_clock=False):
                # evacuate numerator + denominators (frees PSUM banks)
                oc = oc_pool.tile([65, 2, SC], F32, tag="oc")
                nc.vector.tensor_copy(oc[:], op2[:])
                # reshape each [1, 512] sums row to [32, 16] so the
                # reciprocal runs on many DVE lanes instead of one
                rsum = r_pool.tile([64, 16], F32, tag="rsum")
                for hh in range(2):
                    nc.sync.dma_start(
                        rsum[32 * hh: 32 * hh + 32, :],
                        oc[64:65, hh, :].rearrange("o (a n) -> o a n", a=32))
                rrecs = r_pool.tile([64, 16], F32, tag="rrecs")
                nc.vector.reciprocal_approx_fast(rrecs[:], rsum[:])
                rrec = r_pool.tile([1, 2, SC], F32, tag="rrec")
                for hh in range(2):
                    nc.sync.dma_start(
                        rrec[0:1, hh, :].rearrange("o (a n) -> o a n", a=32),
                        rrecs[32 * hh: 32 * hh + 32, :])
                for hh in range(2):
                    bc = bc_pool.tile([64, SC], F32, tag="bc")
                    nc.gpsimd.partition_broadcast(bc[:], rrec[0:1, hh, :])
                    if hh == 0:
                        nc.vector.tensor_tensor(
                            a_tile[0:64, :], oc[0:64, hh, :], bc[:], MUL)
                    else:
                        tb = bc_pool.tile([64, SC], BF16, tag="tb")
                        nc.vector.tensor_tensor(
                            tb[:], oc[0:64, hh, :], bc[:], MUL)
                        # move to partitions 64:128 on the idle SWDGE queue
                        nc.gpsimd.dma_start(a_tile[64:128, :], tb[:])

            def divide_fast(a_tile, op2):
                """Latency-optimized divide for the final pair: PE rank-1
                broadcast of the bf16 denominator row, approx reciprocal,
                no small-DMA roundtrips in the chain."""
                oc = oc_pool.tile([65, 2, SC], BF16, tag="ocf")
                nc.vector.tensor_copy(oc[:], op2[:])
                pbcA = acc_pool.tile([64, SC], F32, tag="acc")
                pbcB = acc_pool.tile([64, SC], F32, tag="acc")
                nc.tensor.matmul(pbcA[:], ones_b[64:65, :], oc[64:65, 0, :],
                                 start=True, stop=True)
                nc.tensor.matmul(pbcB[:], ones_b[64:65, :], oc[64:65, 1, :],
                                 start=True, stop=True)
                rbcA = bc_pool.tile([64, SC], F32, tag="bc")
                rbcB = bc_pool.tile([64, SC], F32, tag="bc")
                nc.vector.reciprocal_approx_fast(rbcA[:], pbcA[:])
                nc.vector.reciprocal_approx_fast(rbcB[:], pbcB[:])
                nc.vector.tensor_tensor(a_tile[0:64, :], oc[0:64, 0, :],
                                        rbcA[:], MUL)
                tb = bc_pool.tile([64, SC], BF16, tag="tb")
                nc.vector.tensor_tensor(tb[:], oc[0:64, 1, :], rbcB[:], MUL)
                nc.gpsimd.dma_start(a_tile[64:128, :], tb[:])

            def final_out_proj(c0, divide_chain, a1):
                """Out-projection of the last chunk: pair-0 accumulations
                start in repurposed score-PSUM slots while the last divide
                is still in flight; the pair-1 A tile's odd head arrives via
                a cross-partition move on the quiet SWDGE queue."""
                osb = o_pool.tile([128, 8, SC], BF16, tag="osb",
                                  name=f"osb{c0}")
                # 4 o_ps slots carved out of the (now idle) score PSUM bufs
                s_a = s_pool.tile([128, 2, SC], F32, tag="s2", name="fop_a")
                s_b = s_pool.tile([128, 2, SC], F32, tag="s2", name="fop_b")
                slots = [s_a[:, 0, :], s_a[:, 1, :], s_b[:, 0, :], s_b[:, 1, :]]

                def mm0(mb, o_ps):
                    nc.tensor.matmul(
                        o_ps, wo_sb[:, 0, mb * 128:(mb + 1) * 128],
                        A_ch[c0][0][:, :], start=True, stop=False)

                def mm1(mb, o_ps):
                    nc.tensor.matmul(
                        o_ps, wo_sb[:, 1, mb * 128:(mb + 1) * 128],
                        a1[:], start=False, stop=True)

                # phase 1: pair-0 accumulation for mb0-5 (depends only on the
                # pair-0 divide, done long ago); divide chain ops interleave,
                # and dummy matmuls keep the PE clock from dropping while the
                # divide's DVE chain runs
                accs = []
                for mb in range(6):
                    if mb < 4:
                        o_ps = slots[mb]
                    else:
                        o_t = acc_pool.tile([128, SC], F32, tag="acc",
                                            name=f"fop_acc{mb}")
                        o_ps = o_t[:]
                    accs.append(o_ps)
                    mm0(mb, o_ps)
                    divide_chain(mb)
                # phase 2: pair-1 accumulation + casts + early stores
                for mb in range(6):
                    mm1(mb, accs[mb])
                    divide_chain(6 + mb)
                    if mb % 2 == 1:
                        nc.scalar.copy(osb[:, mb, :], accs[mb])
                        eng = nc.scalar if mb == 1 else nc.sync
                        eng.dma_start(out[:, c0, mb - 1:mb + 1, :],
                                      osb[:, mb - 1:mb + 1, :])
                    else:
                        nc.vector.tensor_copy(osb[:, mb, :], accs[mb])
                # phase 3: mb6-7 through the regular acc pool
                for mb in range(6, 8):
                    o_ps = acc_pool.tile([128, SC], F32, tag="acc")
                    mm0(mb, o_ps[:])
                    mm1(mb, o_ps[:])
                    if mb % 2 == 1:
                        nc.scalar.copy(osb[:, mb, :], o_ps[:])
                    else:
                        nc.vector.tensor_copy(osb[:, mb, :], o_ps[:])
                nc.sync.dma_start(out[:, c0, 6:8, :], osb[:, 6:8, :])

            # chunk 0: run only what attention pair 0 needs eagerly (KV, K
            # RoPE, Q-m0 + its RoPE); V transposes and Q-m1 go to the filler
            # so the first scores matmul fires as soon as the DMAs land
            proj_kv_items(0, X_ch[0])
            kvi = [filler.popleft() for _ in range(len(filler))]
            proj_q_items(0, X_ch[0], ms=(0,), rope_eng=nc.gpsimd)
            qi0 = [filler.popleft() for _ in range(len(filler))]
            # the KV->K-RoPE->dup chain gates the first scores matmul pair;
            # emit it high-priority so the compile-time scheduler doesn't
            # push the KV projection behind Q work
            with tc.high_priority():
                for it in (kvi[0], kvi[1], kvi[2], kvi[3]):
                    it()
            for it in (qi0[0], qi0[1], qi0[2], qi0[3]):
                it()
            # deferred loads: wq-m1 (needed mid-pair-0), wo (needed at the
            # first out-projection, a chunk later)
            nc.scalar.dma_start(wq_m[1][:], wqP[1])
            nc.scalar.dma_start(wo_sb[:], woP[:])
            filler.extend(kvi[4:7])                   # V cast + transposes
            proj_q_items(0, X_ch[0], ms=(1,))         # Q-m1 as filler
            xn = None
            pending_outproj = None
            for c0 in range(NCH):
                a_pair = [ach_pool.tile([128, SC], BF16, tag="a",
                                        name=f"a_c{c0}p{i}")
                          for i in range(2)]
                A_ch.append(a_pair)
                if c0 == 0:
                    X_ch.append(load_x(1))
                if c0 + 1 < NCH:
                    xn = X_ch[c0 + 1]
                    proj_q_items(c0 + 1, xn)
                op0 = attention_pair(c0, 0)
                if c0 == 0:
                    load_rope_rest()
                divide_pair(a_pair[0], op0, pad_clock=(c0 <= 1))
                if c0 + 1 < NCH:
                    proj_kv_items(c0 + 1, xn)
                held = []
                if pending_outproj is not None:
                    # ACT-assisted casts only while the ACT engine has slack
                    # (pairs 2-5); the last chunk's attention is exp-bound,
                    # so chunk 2's out-projection keeps its casts on DVE
                    held = out_proj_items(pending_outproj,
                                          hold_tail=(c0 == NCH - 1),
                                          act_casts=(c0 < NCH - 1))
                op1 = attention_pair(c0, 1)
                for it in held:
                    it()
                if c0 == NCH - 1:
                    # final pair: latency-optimized divide interleaved with
                    # the final out-projection's pair-0 accumulation; both
                    # head-halves stay on partitions 0:64
                    steps = deque()
                    a1 = ach_pool.tile([128, SC], BF16, tag="a",
                                        name="a1f")

                    def queue_divide():
                        # only the denominator rows are evacuated (1-lane
                        # casts, split per head so head 0's chain starts
                        # sooner); the numerator multiplies read op1's PSUM
                        # directly -- nothing else needs those banks after
                        ocd = oc_pool.tile([1, 2, SC], BF16, tag="ocd")
                        nc.vector.tensor_copy(ocd[:], op1[64:65, :, :])
                        pbcA = acc_pool.tile([64, SC], F32, tag="acc")
                        pbcB = acc_pool.tile([64, SC], F32, tag="acc")
                        rbcA = bc_pool.tile([64, SC], F32, tag="bc")
                        rbcB = bc_pool.tile([64, SC], F32, tag="bc")
                        # dummy matmuls keep the PE clock up while the
                        # divide's DVE chain runs; reading ocd pins them in
                        # this window (the scheduler can't hoist them early),
                        # and they land in pbc banks that the real broadcasts
                        # overwrite right after
                        for i in range(12):
                            tgt = pbcA if i % 2 == 0 else pbcB
                            nc.tensor.matmul(tgt[:, 0:128], ones_b[0:1, :],
                                             ocd[0:1, 0, 0:128],
                                             start=True, stop=True)

                        def st1():
                            nc.tensor.matmul(pbcA[:], ones_b[0:1, :],
                                             ocd[0:1, 0, :],
                                             start=True, stop=True)
                            nc.tensor.matmul(pbcB[:], ones_b[0:1, :],
                                             ocd[0:1, 1, :],
                                             start=True, stop=True)

                        def st2():
                            nc.vector.reciprocal_approx_fast(rbcA[:], pbcA[:])
                            nc.vector.tensor_tensor(
                                a1[0:64, :], op1[0:64, 0, :], rbcA[:], MUL)
                            # WAR on pbcA orders these after the reciprocal:
                            # they fill the PE during the divide's DVE ops
                            for _ in range(6):
                                nc.tensor.matmul(pbcA[:, 0:128],
                                                 ones_b[0:1, :],
                                                 ocd[0:1, 0, 0:128],
                                                 start=True, stop=True)

                        def st3():
                            nc.vector.reciprocal_approx_fast(rbcB[:], pbcB[:])
                            tbf = bc_pool.tile([64, SC], BF16, tag="tb",
                                               name="tbf")
                            nc.vector.tensor_tensor(
                                tbf[:], op1[0:64, 1, :], rbcB[:], MUL)
                            # rings are quiet here; the SWDGE queue keeps
                            # this off the store-carrying HWDGE queues
                            nc.gpsimd.dma_start(a1[64:128, :], tbf[:])
                            for _ in range(6):
                                nc.tensor.matmul(pbcB[:, 0:128],
                                                 ones_b[0:1, :],
                                                 ocd[0:1, 0, 0:128],
                                                 start=True, stop=True)

                        steps.extend([st1, st2, st3])

                    queue_divide()

                    def divide_chain(i, steps=steps):
                        if steps:
                            steps.popleft()()

                    drain_filler()
                    final_out_proj(c0, divide_chain, a1)
                else:
                    divide_pair(a_pair[1], op1, pad_clock=(c0 <= 1))
                    if c0 + 2 < NCH:
                        X_ch.append(load_x(c0 + 2))
                    # Q/KV of chunk c0+1 must be ready before attention starts
                    drain_filler()
                    pending_outproj = c0

    nc.compile()
    return nc


def shard_inputs(x, wq, wk, wv, wo, freqs_cos, freqs_sin):
    """Build the 8 per-core input maps (host-side layout prep)."""
    x = np.ascontiguousarray(np.asarray(x, dtype=np.float32))
    wq = np.asarray(wq, dtype=np.float32)
    wk = np.asarray(wk, dtype=np.float32)
    wv = np.asarray(wv, dtype=np.float32)
    wo = np.asarray(wo, dtype=np.float32)
    cos = np.asarray(freqs_cos, dtype=np.float32)   # [S, 32]
    sin = np.asarray(freqs_sin, dtype=np.float32)
    bf = ml_dtypes.bfloat16

    rope_cos = np.repeat(cos.T, 2, axis=0)          # [64, S]
    rope_sin = np.repeat(sin.T, 2, axis=0)
    rope_sin[0::2, :] *= -1.0                       # row 2i: -sin_i, 2i+1: +sin_i
    rope_cos = np.ascontiguousarray(
        np.concatenate([rope_cos, rope_cos], 0)).astype(bf)
    rope_sin = np.ascontiguousarray(
        np.concatenate([rope_sin, rope_sin], 0)).astype(bf)

    # 0/1 keep-mask for a 128x128 diagonal block: 0 where k > q (strict
    # upper triangle), applied to p2 post-exp
    kk = np.arange(128)[:, None]
    qq = np.arange(128)[None, :]
    tri = np.where(kk > qq, np.float32(0.0), np.float32(1.0))
    triM = np.ascontiguousarray(
        np.broadcast_to(tri[:, None, :], (128, 2, 128))).astype(bf)

    in_maps = []
    for core in range(N_CORES):
        b, r = divmod(core, TPG)
        xT = x[b].T                                               # [DM, S]
        # pack so each SBUF partition line is one contiguous DRAM run
        xPm = np.ascontiguousarray(
            xT.reshape(8, 128, NCH, SC).transpose(1, 2, 0, 3))    # [128,NCH,8,SC]
        wq_s = wq[r * KFEAT:(r + 1) * KFEAT]                      # [256, DM]
        wk_s = wk[r * HD:(r + 1) * HD]                            # [64, DM]
        wv_s = wv[r * HD:(r + 1) * HD]
        wkvT = np.concatenate([wk_s, wv_s], axis=0).T             # [DM, 128]
        wqT = wq_s.T                                              # [DM, 256]
        woT = wo[:, r * KFEAT:(r + 1) * KFEAT].T                  # [256, DM]
        wqPm = np.ascontiguousarray(
            wqT.reshape(8, 128, 2, 128).transpose(2, 1, 0, 3))    # [2, 128, 8, 128]
        wkvPm = np.ascontiguousarray(
            wkvT.reshape(8, 128, 128).transpose(1, 0, 2))         # [128, 8, 128]
        woPm = np.ascontiguousarray(
            woT.reshape(2, 128, DM).transpose(1, 0, 2))           # [128, 2, 1024]
        in_maps.append({
            "xP": xPm.astype(bf),
            "wqP": wqPm.astype(bf),
            "wkvP": wkvPm.astype(bf),
            "woP": woPm.astype(bf),
            "ropeCos": rope_cos,
            "ropeSin": rope_sin,
            "triM": triM,
        })
    return in_maps


def unshard(results):
    """Sum TP partials per batch, unpack, and transpose to [B, S, DM]."""
    out = np.empty((B, S, DM), dtype=np.float32)
    for b in range(B):
        acc = results[b * TPG]["out"].astype(np.float32)
        for r in range(1, TPG):
            acc = acc + results[b * TPG + r]["out"].astype(np.float32)
        # [128, NCH, 8, SC] -> [DM, S]: row (mb*128+p), col (c*SC+n)
        full = acc.transpose(2, 0, 1, 3).reshape(DM, S)
        out[b] = full.T
    return out


def kernel(**inputs):
    global LAST_RESULTS, _NC_CACHE
    if _NC_CACHE is None:
        _NC_CACHE = build_nc()
    in_maps = shard_inputs(**inputs)
    LAST_RESULTS = run_bass_kernel_spmd(_NC_CACHE, in_maps, list(range(N_CORES)))
    return unshard(LAST_RESULTS.results)
